# revision 15
# baseline (speedup 1.0000x reference)
"""BiAttention Trainium2 kernel.

Reference math (per batch; n = m = 1024, d = 512):
    sim[n,m] = (x1*w3) @ x2.T + s1[n] + s2[m] + bias,  s1 = x1@w1, s2 = x2@w2
    row softmax over m with x2-masked columns dropped -> attn_a = P_row @ x2
    col softmax over n with x1-masked rows dropped    -> q2c = P_col.T @ x1
    attn_b = P_row @ q2c

Kernel formulation (softmax is shift-invariant, so each direction only needs
the logit terms that vary along its own axis):
    ER^T[m,n] = exp(s3^T[m,n] + lane2[m]),  lane2 = s2 + bias + NEG*x2_mask
        (s1[n] cancels in the row softmax; lane2 is per-partition in the
         [m,n] layout -> applied as the ACT exp bias)
    EC[n,m]   = exp(s3[n,m] + s1[n])
        (s2/bias/colmask cancel in the col softmax; s1 per-partition bias)
    rowsum[n] = sum_m EC[n,m]*exp(lane2[m]) / exp(s1[n])   (DVE mult+reduce
        against a broadcast exp(lane2) row; exp(s1) folded into 1/rowsum)
    colsum'[m] = sum_n ER^T[m,n]*B[n],  B = exp(s1)*keep1   (x1 row mask
        lives in B); q2c scale = exp(lane2[m]) / colsum'[m]
    attn_a = (ER^T.T @ x2) / rowsum
    q2c    = (EC.T @ (keep1*x1)) * exp(lane2)/colsum'
    attn_b = (ER^T.T @ q2c) / rowsum
Masked x2 columns give colsum' = 0 and exp(lane2) = 0 -> q2c rows forced to 0,
which is fine: their attn_b weights are exactly 0 (exp underflow), matching
the reference where those q2c rows are finite but multiplied by 0.

Implementation notes:
  - exp() without max-subtraction: logits are O(+-8) here, and masking is
    additive -30000 so exp underflows to exactly 0.
  - Matmuls run in fp16 (10-bit mantissa; 2-byte operands stream at 1
    cycle/row with fast-weight-load). PSUM accumulation is fp32.
  - x1^T*w3, x2^T, keep1*x1 and fp16 casts are prepared host-side (pure
    layout/elementwise marshaling); all O(n*m*d) compute runs on device.
  - Sharding: data-parallel over batch, 2 batches per core, 8 cores.
"""

import os
import sys

import numpy as np

for _p in ("/opt/trn_rl_repo",):
    if _p not in sys.path:
        sys.path.append(_p)

import concourse.bass as bass
import concourse.mybir as mybir
import concourse.tile as tile
from concourse import bass_utils
from concourse.bass import ds, ts
from concourse.tile import ScopedClock

NCORES = 8
B, N, M, D = 16, 1024, 1024, 512
BPC = B // NCORES  # batches per core
NEG = -30000.0  # additive mask: exp(x + NEG) == 0 for |x| < ~100

F32 = mybir.dt.float32
F32R = mybir.dt.float32r
F16 = mybir.dt.float16
BF16 = mybir.dt.bfloat16

MM_DT = {"f32r": F32R, "f16": F16, "bf16": BF16}[os.environ.get("BIATT_MM_DT", "f16")]
_MM_NP = {F32R: np.float32, F16: np.float16}
if MM_DT == BF16:
    import ml_dtypes as _mld

    _MM_NP[BF16] = _mld.bfloat16
MM_NP = _MM_NP[MM_DT]

NT = N // 128  # 8 n-tiles
MT = M // 128  # 8 m-tiles
DC = D // 128  # 4 d-chunks
NH = N // 512  # 2 n-halves (PSUM-bank-sized slabs)
MH = M // 512  # 2 m-halves

# ---------------------------------------------------------------------------
# Workarounds for this walrus build: at most ONE sync wait per instruction.
# ---------------------------------------------------------------------------

_ctr = [0]


def _split_multi_waits(nc):
    """Move extra sync waits onto same-engine InstNoOp carriers inserted
    immediately before the over-subscribed instruction."""
    for f in nc.m.functions:
        for bb in f.blocks:
            insts = bb.instructions
            i = 0
            while i < len(insts):
                inst = insts[i]
                si = getattr(inst, "sync_info", None)
                if si is not None and len(si.on_wait) > 1:
                    waits = list(si.on_wait)
                    carriers = []
                    for w in waits[:-1]:
                        _ctr[0] += 1
                        carriers.append(
                            mybir.InstNoOp(
                                name=f"I-waitsplit-{_ctr[0]}",
                                engine=inst.engine,
                                bass_nofuse=True,
                                sync_info=mybir.SyncInfo(on_wait=[w], on_update=[]),
                            )
                        )
                    inst.sync_info = mybir.SyncInfo(
                        on_wait=[waits[-1]], on_update=list(si.on_update)
                    )
                    insts[i:i] = carriers
                    i += len(carriers)
                i += 1


def _patched_drain_and_barrier(self, tick_clock, wait_clock):
    """TileContext tail drain: carry the global-clock waits on SP nops (the
    Drain opcode can't encode sync waits in this walrus build)."""
    nc = self.nc
    nop_inst = nc.sync.nop(nofuse=True)
    wait_clock.add_sem_waits(nop_inst.ins, ScopedClock({None: tick_clock.global_clock}))
    waits = list(nop_inst.ins.sync_info.on_wait)
    if len(waits) > 1:
        nop_inst.ins.sync_info = mybir.SyncInfo(on_wait=[waits[0]], on_update=[])
        for w in waits[1:]:
            extra = nc.sync.nop(nofuse=True)
            extra.ins.sync_info = mybir.SyncInfo(on_wait=[w], on_update=[])
    nc.sync.drain()
    nc.all_engine_barrier()
    assert self.sems is not None
    popped = nc._tile_sem_poison_stack.pop()
    assert popped is self._sem_poison
    nc.clear_and_free_semaphores(list(self.sems.allocated().values()))
    nc.all_engine_barrier()


tile.TileContext._drain_and_barrier = _patched_drain_and_barrier

# ---------------------------------------------------------------------------
# Kernel build
# ---------------------------------------------------------------------------

_cache = {}


def _build():
    nc = bass.Bass("TRN2", target_bir_lowering=False, debug=False)

    # transposed operands for the similarity matmuls (d on partitions)
    x1td = nc.dram_tensor("x1t", [BPC, D, N], MM_DT, kind="ExternalInput").ap()
    x2td = nc.dram_tensor("x2t", [BPC, D, M], MM_DT, kind="ExternalInput").ap()
    # natural-layout rhs operands
    x1md = nc.dram_tensor("x1m", [BPC, N, D], MM_DT, kind="ExternalInput").ap()  # keep1*x1
    x2d = nc.dram_tensor("x2", [BPC, M, D], MM_DT, kind="ExternalInput").ap()
    # fused per-row/col logit vectors, [128, 4*NT] per-partition layout (fp32):
    # columns = [s1 | s2+bias+NEG*m2 | exp(s1) | exp(s2+bias)*keep2]
    lvecd = nc.dram_tensor("lvec", [BPC, 128, 4 * NT], F32, kind="ExternalInput").ap()
    # broadcast row sources (fp16): [exp(s1)*keep1 (N) | exp(s2+bias)*keep2 (M)]
    bcd = nc.dram_tensor("bc", [BPC, N + M], MM_DT, kind="ExternalInput").ap()
    oad = nc.dram_tensor("attn_a", [BPC, N, D], F32, kind="ExternalOutput").ap()
    obd = nc.dram_tensor("attn_b", [BPC, N, D], F32, kind="ExternalOutput").ap()

    EXP = mybir.ActivationFunctionType.Exp
    AX = mybir.AxisListType.X

    with tile.TileContext(nc) as tc:
        with (
            tc.tile_pool(name="xin", bufs=2) as xin,
            tc.tile_pool(name="amat", bufs=2) as amat,
            tc.tile_pool(name="emat", bufs=2) as emat,
            tc.tile_pool(name="qmat", bufs=2) as qmat,
            tc.tile_pool(name="small", bufs=2) as small,
            tc.tile_pool(name="tmp", bufs=3) as tmp,
            tc.tile_pool(name="ostage", bufs=4) as ostage,
            tc.tile_pool(name="mm_ps", bufs=4, space="PSUM") as mm_ps,
            tc.tile_pool(name="acc_ps", bufs=3, space="PSUM") as acc_ps,
        ):
            # trigger the ACT exp table load while the first DMAs are in
            # flight (the first real exp would otherwise pay ~2.7us mid-loop)
            warm = small.tile([128, 2], F32, tag="warm")
            nc.vector.memset(warm[:], 0.0)
            nc.scalar.activation(out=warm[:], in_=warm[:], func=EXP)

            for b in range(BPC):
                # ---- loads ------------------------------------------------
                lvec = small.tile([128, 4 * NT], F32, tag="lvec")
                nc.sync.dma_start(out=lvec[:], in_=lvecd[b])
                l1n = lvec[:, 0:NT]
                l2m = lvec[:, NT : NT + MT]
                es1 = lvec[:, NT + MT : 2 * NT + MT]
                el2 = lvec[:, 2 * NT + MT : 2 * NT + 2 * MT]
                # A1/A2 feed the first matmuls: split the loads across the
                # sync and gpsimd queues so triggers issue in parallel
                A1 = amat.tile([128, DC, N], MM_DT, tag="A1")  # w3*x1^T
                A2 = amat.tile([128, DC, M], MM_DT, tag="A2")  # x2^T
                for c in range(DC):
                    nc.sync.dma_start(out=A2[:, c, :], in_=x2td[b, ts(c, 128), :])
                    nc.gpsimd.dma_start(out=A1[:, c, :], in_=x1td[b, ts(c, 128), :])
                BCT = small.tile([128, N + M], MM_DT, tag="BCT")
                nc.gpsimd.dma_start(
                    out=BCT[:], in_=bcd[b][None, :].to_broadcast([128, N + M])
                )
                BV = BCT[:, 0:N]
                EV = BCT[:, N : N + M]
                X1M = xin.tile([128, NT, D], MM_DT, tag="X1M")
                X2 = xin.tile([128, MT, D], MM_DT, tag="X2")
                for t in range(NT):
                    nc.gpsimd.dma_start(out=X1M[:, t, :], in_=x1md[b, ts(t, 128), :])
                for t in range(MT):
                    nc.gpsimd.dma_start(out=X2[:, t, :], in_=x2d[b, ts(t, 128), :])

                # ---- ER^T = exp(s3^T + lane2[m])  [m-part, n-free] --------
                ET = emat.tile([128, MT, N], MM_DT, tag="ET")
                for mt in range(MT):
                    for nh in range(NH):
                        ps = mm_ps.tile([128, 512], F32, tag="mm")
                        for c in range(DC):
                            nc.tensor.matmul(
                                ps[:],
                                A2[:, c, ts(mt, 128)],
                                A1[:, c, ds(512 * nh, 512)],
                                start=(c == 0),
                                stop=(c == DC - 1),
                            )
                        nc.scalar.activation(
                            out=ET[:, mt, ds(512 * nh, 512)],
                            in_=ps[:],
                            func=EXP,
                            bias=l2m[:, mt : mt + 1],
                        )
                # ---- EC = exp(s3 + s1[n])  [n-part, m-free] ---------------
                EC = emat.tile([128, NT, M], MM_DT, tag="EC")
                for nt in range(NT):
                    for mh in range(MH):
                        ps = mm_ps.tile([128, 512], F32, tag="mm")
                        for c in range(DC):
                            nc.tensor.matmul(
                                ps[:],
                                A1[:, c, ts(nt, 128)],
                                A2[:, c, ds(512 * mh, 512)],
                                start=(c == 0),
                                stop=(c == DC - 1),
                            )
                        nc.scalar.activation(
                            out=EC[:, nt, ds(512 * mh, 512)],
                            in_=ps[:],
                            func=EXP,
                            bias=l1n[:, nt : nt + 1],
                        )

                # ---- denominators off the PE ------------------------------
                # 1/rowsum[n] = exp(s1[n]) / sum_m EC[n,m]*EV[m]
                RR = small.tile([128, NT], F32, tag="RR")
                rtmp = small.tile([128, NT], F32, tag="rtmp")
                for nt in range(NT):
                    scr = tmp.tile([128, M], MM_DT, tag="scr")
                    nc.vector.tensor_mul(scr[:], EC[:, nt, :], EV[:])
                    nc.vector.reduce_sum(out=rtmp[:, nt : nt + 1], in_=scr[:], axis=AX)
                    nc.vector.reciprocal(rtmp[:, nt : nt + 1], rtmp[:, nt : nt + 1])
                    nc.vector.tensor_scalar_mul(
                        RR[:, nt : nt + 1], es1[:, nt : nt + 1], rtmp[:, nt : nt + 1]
                    )
                # q2c scale[m] = exp(lane2[m]) / (sum_n ER^T[m,n]*BV[n] + eps)
                CR = small.tile([128, MT], F32, tag="CR")
                ctmp = small.tile([128, MT], F32, tag="ctmp")
                for mc in range(MT):
                    scr = tmp.tile([128, N], MM_DT, tag="scr")
                    nc.vector.tensor_mul(scr[:], ET[:, mc, :], BV[:])
                    nc.vector.reduce_sum(out=ctmp[:, mc : mc + 1], in_=scr[:], axis=AX)
                    nc.vector.tensor_scalar_add(
                        ctmp[:, mc : mc + 1], ctmp[:, mc : mc + 1], 1e-30
                    )
                    nc.vector.reciprocal(ctmp[:, mc : mc + 1], ctmp[:, mc : mc + 1])
                    nc.vector.tensor_scalar_mul(
                        CR[:, mc : mc + 1], el2[:, mc : mc + 1], ctmp[:, mc : mc + 1]
                    )

                # ---- attn_a = (ER^T.T @ x2) / rowsum ----------------------
                for nt in range(NT):
                    aps = acc_ps.tile([128, 512], F32, tag="acc")
                    for mc in range(MT):
                        nc.tensor.matmul(
                            aps[:],
                            ET[:, mc, ts(nt, 128)],
                            X2[:, mc, :],
                            start=(mc == 0),
                            stop=(mc == MT - 1),
                        )
                    stage = ostage.tile([128, 512], F32, tag="stage")
                    nc.scalar.mul(stage[:], aps[:], RR[:, nt : nt + 1])
                    nc.sync.dma_start(out=oad[b, ts(nt, 128), :], in_=stage[:])

                # ---- q2c = (EC.T @ (keep1*x1)) * CR -----------------------
                Q2C = qmat.tile([128, MT, D], MM_DT, tag="Q2C")
                for mt in range(MT):
                    qps = acc_ps.tile([128, 512], F32, tag="acc")
                    for nc_ in range(NT):
                        nc.tensor.matmul(
                            qps[:],
                            EC[:, nc_, ts(mt, 128)],
                            X1M[:, nc_, :],
                            start=(nc_ == 0),
                            stop=(nc_ == NT - 1),
                        )
                    nc.scalar.mul(Q2C[:, mt, :], qps[:], CR[:, mt : mt + 1])

                # ---- attn_b = (ER^T.T @ q2c) / rowsum ---------------------
                for nt in range(NT):
                    bps = acc_ps.tile([128, 512], F32, tag="acc")
                    for mc in range(MT):
                        nc.tensor.matmul(
                            bps[:],
                            ET[:, mc, ts(nt, 128)],
                            Q2C[:, mc, :],
                            start=(mc == 0),
                            stop=(mc == MT - 1),
                        )
                    stage = ostage.tile([128, 512], F32, tag="stage")
                    nc.scalar.mul(stage[:], bps[:], RR[:, nt : nt + 1])
                    nc.sync.dma_start(out=obd[b, ts(nt, 128), :], in_=stage[:])

    _split_multi_waits(nc)
    return nc


def _get_nc():
    if "nc" not in _cache:
        _cache["nc"] = _build()
    return _cache["nc"]


# ---------------------------------------------------------------------------
# Host entry point
# ---------------------------------------------------------------------------


def _prep(x1, x1_mask, x2, x2_mask, w, bias):
    """Host-side marshaling: layout transposes, fp16 casts, and the tiny
    O(b*(n+m)*d) per-row/col logit vectors."""
    x1 = np.asarray(x1, dtype=np.float32)
    x2 = np.asarray(x2, dtype=np.float32)
    x1_mask = np.asarray(x1_mask, dtype=bool)
    x2_mask = np.asarray(x2_mask, dtype=bool)
    w = np.asarray(w, dtype=np.float32)
    bias_f = float(np.asarray(bias, dtype=np.float32))

    d = x1.shape[-1]
    w1, w2, w3 = w[:d], w[d : 2 * d], w[2 * d :]
    s1 = np.einsum("bnd,d->bn", x1, w1)
    s2 = np.einsum("bmd,d->bm", x2, w2)
    keep1 = np.where(x1_mask, np.float32(0), np.float32(1))
    keep2 = np.where(x2_mask, np.float32(0), np.float32(1))
    lane2 = s2 + bias_f + np.where(x2_mask, np.float32(NEG), np.float32(0))

    x1t = np.ascontiguousarray((x1 * w3).transpose(0, 2, 1).astype(MM_NP))
    x2t = np.ascontiguousarray(x2.transpose(0, 2, 1).astype(MM_NP))
    x1m = np.ascontiguousarray((x1 * keep1[:, :, None]).astype(MM_NP))
    x2c = np.ascontiguousarray(x2.astype(MM_NP))

    def ptile(v):  # [B, L] -> [B, 128, L//128] with v[b, t*128+p] at [b, p, t]
        return np.ascontiguousarray(
            v.reshape(B, -1, 128).transpose(0, 2, 1).astype(np.float32)
        )

    es1 = np.exp(s1)
    el2 = np.exp(s2 + bias_f) * keep2
    lvec = np.concatenate(
        [ptile(s1), ptile(lane2), ptile(es1), ptile(el2)], axis=2
    )
    bc = np.concatenate(
        [(es1 * keep1).astype(MM_NP), el2.astype(MM_NP)], axis=1
    )
    return {
        "x1t": x1t,
        "x2t": x2t,
        "x1m": x1m,
        "x2": x2c,
        "lvec": np.ascontiguousarray(lvec),
        "bc": np.ascontiguousarray(bc),
    }


def _run(x1, x1_mask, x2, x2_mask, w, bias, **run_kwargs):
    full = _prep(x1, x1_mask, x2, x2_mask, w, bias)
    nc = _get_nc()
    in_maps = []
    for core in range(NCORES):
        lo, hi = core * BPC, (core + 1) * BPC
        in_maps.append({k: v[lo:hi] for k, v in full.items()})
    res = bass_utils.run_bass_kernel_spmd(
        nc, in_maps, core_ids=list(range(NCORES)), **run_kwargs
    )
    attn_a = np.concatenate([res.results[c]["attn_a"] for c in range(NCORES)], axis=0)
    attn_b = np.concatenate([res.results[c]["attn_b"] for c in range(NCORES)], axis=0)
    return (attn_a, attn_b), res


def kernel(x1, x1_mask, x2, x2_mask, w, bias):
    out, _ = _run(x1, x1_mask, x2, x2_mask, w, bias)
    return out
